# revision 1
# baseline (speedup 1.0000x reference)
"""Causal multi-head self-attention (B=2, S=2048, D=2048, H=16) on 8 TRN2
NeuronCores.

Sharding: core c -> (batch b = c // 4, head-group hg = c % 4). Each core
computes 4 heads of one batch: QKV projections (tensor-parallel column
slices), RoPE, causal attention, and a partial W_O row-slice projection.
The host sums the 4 partial outputs per batch (replaces the all-reduce).

Layouts (per core):
  xT   [D, S]    x[b] transposed; matmul moving operand / stationary for V
  wqT  [D, 512]  W_Q[hslice].T with per-head deinterleave column permutation
  wkT  [D, 512]  same for W_K
  wvT  [D, 512]  W_V[hslice].T (natural order)
  woT  [512, D]  W_O[:, hslice].T (natural order)
  QT/KT per head [128, S] (transposed, deinterleaved dk order, RoPE applied)
  V per s-tile   [128, 512] (natural [s, dk] order)
  scores computed transposed [k, q] so exp tiles feed the AV matmul as the
  moving operand with V tiles stationary; softmax denominators via DVE
  accumulation + a ones-matmul that also broadcasts across partitions.

All matmuls use float32r (full-rate fp32 streaming) with N=512.
Projection phases run D-tile-outer: one xT tile streams through 8 (Q+K)
PSUM accumulation chains so only a few xT tiles are SBUF-live at a time.
"""
import sys

if "/opt/trn_rl_repo" not in sys.path:
    sys.path.insert(0, "/opt/trn_rl_repo")

import math
import numpy as np

import concourse.bass as bass
import concourse.mybir as mybir
import concourse.tile as tile
from concourse import bacc
from concourse.bass_utils import run_bass_kernel_spmd

B, S, D, H = 2, 2048, 2048, 16
DK = D // H            # 128
THETA = 10000.0
N_CORES = 8
NH = 4                 # heads per core
DKL = NH * DK          # 512 local head dims
P = 128
SBK = 512              # s-block (matmul N)
NDT = D // P           # 16 D-tiles
NST = S // P           # 16 s-tiles
NSB = S // SBK         # 4 s-blocks
NQB = S // SBK         # 4 q-blocks per head
NKT = S // P           # 16 k-tiles

F32 = mybir.dt.float32
F32R = mybir.dt.float32r

_CACHE = {}


def build_program(repeat=1):
    """Build the single-core SPMD program. repeat>1 wraps the body in a
    hardware loop (timing only)."""
    nc = bacc.Bacc("TRN2", target_bir_lowering=False, debug=False)

    xT = nc.dram_tensor("xT", [D, S], F32R, kind="ExternalInput").ap()
    wqT = nc.dram_tensor("wqT", [D, DKL], F32R, kind="ExternalInput").ap()
    wkT = nc.dram_tensor("wkT", [D, DKL], F32R, kind="ExternalInput").ap()
    wvT = nc.dram_tensor("wvT", [D, DKL], F32R, kind="ExternalInput").ap()
    woT = nc.dram_tensor("woT", [DKL, D], F32R, kind="ExternalInput").ap()
    cs1 = nc.dram_tensor("cs1", [P, S], F32, kind="ExternalInput").ap()
    cs2 = nc.dram_tensor("cs2", [P, S], F32, kind="ExternalInput").ap()
    masks = nc.dram_tensor("masks", [P, NH * SBK], F32, kind="ExternalInput").ap()
    out = nc.dram_tensor("out", [S, D], F32, kind="ExternalOutput").ap()

    inv_sqrt_dk = 1.0 / math.sqrt(DK)

    with tile.TileContext(nc) as tc:
        if True:

            def phase_a(qt, kt_):
                xTr = xT.rearrange("(dt p) s -> p dt s", p=P)
                with tc.tile_pool(name="wqk", bufs=1) as wqk, \
                     tc.tile_pool(name="csp", bufs=2) as csp, \
                     tc.tile_pool(name="xta", bufs=2) as xtp, \
                     tc.tile_pool(name="ropet", bufs=2) as ropep, \
                     tc.tile_pool(name="psa", bufs=8, space="PSUM") as psa:
                    wq_t = wqk.tile([P, NDT, DKL], F32R, name="wqs")
                    wk_t = wqk.tile([P, NDT, DKL], F32R, name="wks")
                    wqTr = wqT.rearrange("(dt p) n -> p dt n", p=P)
                    wkTr = wkT.rearrange("(dt p) n -> p dt n", p=P)
                    for sb in range(NSB):
                        cs1_t = csp.tile([P, SBK], F32, tag="cs1", name=f"cs1_{sb}")
                        cs2_t = csp.tile([P, SBK], F32, tag="cs2", name=f"cs2_{sb}")
                        pps = [psa.tile([P, SBK], F32, tag="pa", name=f"pa{sb}_{i}")
                               for i in range(2 * NH)]
                        for half in range(2):
                            hsl = slice(half * (NDT // 2), (half + 1) * (NDT // 2))
                            xs = xtp.tile([P, NDT // 2, SBK], F32R, tag="xt",
                                          name=f"xa{sb}_{half}")
                            nc.sync.dma_start(
                                out=xs,
                                in_=xTr[:, hsl, sb * SBK:(sb + 1) * SBK])
                            if sb == 0:
                                nc.sync.dma_start(out=wq_t[:, hsl, :], in_=wqTr[:, hsl, :])
                                nc.sync.dma_start(out=wk_t[:, hsl, :], in_=wkTr[:, hsl, :])
                            if half == 1:
                                nc.sync.dma_start(out=cs1_t, in_=cs1[:, sb * SBK:(sb + 1) * SBK])
                                nc.sync.dma_start(out=cs2_t, in_=cs2[:, sb * SBK:(sb + 1) * SBK])
                            for dd in range(NDT // 2):
                                d = half * (NDT // 2) + dd
                                for w_i, wt in enumerate((wq_t, wk_t)):
                                    for dkb in range(NH):
                                        nc.tensor.matmul(
                                            pps[w_i * NH + dkb][:],
                                            wt[:, d, dkb * P:(dkb + 1) * P], xs[:, dd, :],
                                            start=(d == 0), stop=(d == NDT - 1))
                        for w_i, dst in enumerate((qt, kt_)):
                            for dkb in range(NH):
                                pp = pps[w_i * NH + dkb]
                                # one ACT copy frees the PSUM bank; RoPE then
                                # reads SBUF only. rows 0:64 are x1, 64:128 x2.
                                qf = ropep.tile([P, SBK], F32, tag="qf")
                                nc.scalar.copy(qf[:], pp[:])
                                swp = ropep.tile([P, SBK], F32, tag="swp")
                                nc.scalar.copy(swp[0:P // 2], qf[P // 2:P])
                                nc.scalar.copy(swp[P // 2:P], qf[0:P // 2])
                                t1 = ropep.tile([P, SBK], F32, tag="t1")
                                nc.vector.tensor_mul(t1[:], qf[:], cs1_t[:])
                                t2 = ropep.tile([P, SBK], F32, tag="t2")
                                nc.vector.tensor_mul(t2[:], swp[:], cs2_t[:])
                                nc.vector.tensor_add(
                                    dst[dkb][:, sb * SBK:(sb + 1) * SBK], t1[:], t2[:])

            def phase_b(vt):
                xTr = xT.rearrange("(dt p) s -> p dt s", p=P)
                with tc.tile_pool(name="wvp", bufs=1) as wvp, \
                     tc.tile_pool(name="xtb", bufs=2) as xtpb, \
                     tc.tile_pool(name="psb", bufs=4, space="PSUM") as psb:
                    wv_t = wvp.tile([P, NDT, DKL], F32R, name="wvs")
                    wvTr = wvT.rearrange("(dt p) n -> p dt n", p=P)
                    for sb in range(NSB):
                        pvs = [psb.tile([P, DKL], F32, tag="pb", name=f"pb{sb}_{j}")
                               for j in range(SBK // P)]
                        for half in range(2):
                            hsl = slice(half * (NDT // 2), (half + 1) * (NDT // 2))
                            xs = xtpb.tile([P, NDT // 2, SBK], F32R, tag="xtb",
                                           name=f"xb{sb}_{half}")
                            nc.sync.dma_start(
                                out=xs,
                                in_=xTr[:, hsl, sb * SBK:(sb + 1) * SBK])
                            if sb == 0:
                                nc.sync.dma_start(out=wv_t[:, hsl, :], in_=wvTr[:, hsl, :])
                            for dd in range(NDT // 2):
                                d = half * (NDT // 2) + dd
                                for j in range(SBK // P):
                                    nc.tensor.matmul(
                                        pvs[j][:], xs[:, dd, j * P:(j + 1) * P], wv_t[:, d, :],
                                        start=(d == 0), stop=(d == NDT - 1))
                        for j in range(SBK // P):
                            st = sb * (SBK // P) + j
                            nc.scalar.copy(vt[st][:], pvs[j][:])

            def phase_cd(qt, kt_, vt, ot):
                # Attention (qb-outer, heads inner) interleaved with the
                # output projection for the finished q-block, so PE-dense
                # O-proj matmuls fill attention's ACT/DVE-bound stretches.
                with tc.tile_pool(name="maskp", bufs=1) as maskp, \
                     tc.tile_pool(name="wop", bufs=1) as wop, \
                     tc.tile_pool(name="expp", bufs=4) as expp, \
                     tc.tile_pool(name="saccp", bufs=2) as saccp, \
                     tc.tile_pool(name="recipp", bufs=2) as recipp, \
                     tc.tile_pool(name="stage", bufs=2) as stage, \
                     tc.tile_pool(name="pssc", bufs=2, space="PSUM") as pssc, \
                     tc.tile_pool(name="psav", bufs=2, space="PSUM") as psav, \
                     tc.tile_pool(name="psden", bufs=2, space="PSUM") as psden, \
                     tc.tile_pool(name="pso", bufs=2, space="PSUM") as pso:
                    mask_t = maskp.tile([P, NH * SBK], F32)
                    nc.sync.dma_start(out=mask_t, in_=masks)
                    ones_f = maskp.tile([P, P], F32)
                    nc.vector.memset(ones_f, 1.0)
                    ones_r = maskp.tile([P, P], F32R)
                    nc.vector.tensor_copy(ones_r, ones_f)
                    wo_t = wop.tile([P, NH, S], F32R, name="wos")
                    nc.sync.dma_start(out=wo_t, in_=woT.rearrange("(hh p) n -> p hh n", p=P))

                    def attend_main(h, qb):
                        nkt = 4 * qb + 4
                        av = psav.tile([P, SBK], F32, tag="av", name=f"av{h}_{qb}")
                        sacc = saccp.tile([P, SBK], F32R, tag="sacc", name=f"sacc{h}_{qb}")
                        prev = None
                        for kt in range(nkt):
                            sc = pssc.tile([P, SBK], F32, tag="sc", name=f"sc{h}_{qb}_{kt}")
                            nc.tensor.matmul(
                                sc[:], kt_[h][:, kt * P:(kt + 1) * P],
                                qt[h][:, qb * SBK:(qb + 1) * SBK],
                                start=True, stop=True)
                            e = expp.tile([P, SBK], F32R, tag="e", name=f"e{h}_{qb}_{kt}")
                            nc.scalar.activation(
                                e[:], sc[:], mybir.ActivationFunctionType.Exp,
                                scale=inv_sqrt_dk)
                            r = kt - 4 * qb
                            if r >= 0:  # diagonal-straddling tile: mask
                                em = expp.tile([P, SBK], F32R, tag="e", name=f"em{h}_{qb}_{kt}")
                                nc.vector.tensor_mul(
                                    em[:], e[:], mask_t[:, r * SBK:(r + 1) * SBK])
                                e = em
                            if kt == 0:
                                nc.vector.tensor_copy(sacc[:], e[:])
                            else:
                                nc.vector.tensor_add(sacc[:], sacc[:], e[:])
                            if prev is not None:
                                pkt, pe = prev
                                nc.tensor.matmul(
                                    av[:], vt[pkt][:, h * P:(h + 1) * P], pe[:],
                                    start=(pkt == 0), stop=False)
                            prev = (kt, e)
                        pkt, pe = prev
                        nc.tensor.matmul(
                            av[:], vt[pkt][:, h * P:(h + 1) * P], pe[:],
                            start=(pkt == 0), stop=True)
                        return av, sacc

                    def attend_finish(h, qb, av, sacc):
                        den = psden.tile([P, SBK], F32, tag="den", name=f"den{h}_{qb}")
                        nc.tensor.matmul(den[:], ones_r[:], sacc[:],
                                         start=True, stop=True)
                        recip = recipp.tile([P, SBK], F32, tag="recip", name=f"rc{h}_{qb}")
                        nc.vector.reciprocal_approx_fast(out=recip[:], in_=den[:])
                        nc.vector.tensor_mul(
                            ot[h][:, qb * SBK:(qb + 1) * SBK], av[:], recip[:])

                    def oproj(st):
                        for db in range(NSB):
                            po = pso.tile([P, SBK], F32, tag="po", name=f"po{st}_{db}")
                            for hh in range(NH):
                                nc.tensor.matmul(
                                    po[:], ot[hh][:, st * P:(st + 1) * P],
                                    wo_t[:, hh, db * SBK:(db + 1) * SBK],
                                    start=(hh == 0), stop=(hh == NH - 1))
                            og = stage.tile([P, SBK], F32, tag="og", name=f"og{st}_{db}")
                            nc.scalar.copy(og[:], po[:])
                            nc.sync.dma_start(
                                out=out[st * P:(st + 1) * P, db * SBK:(db + 1) * SBK],
                                in_=og[:])

                    # one-unit software pipeline: finish(u-1) emitted after
                    # main(u) so PE never stalls on the sacc chain, and the
                    # q-block's O-projection fills attention's ACT/DVE stretches.
                    units = [(qb, h) for qb in range(NQB) for h in range(NH)]
                    pending = None
                    for qb, h in units:
                        st_ = attend_main(h, qb)
                        if pending is not None:
                            pqb, ph, pav, psacc = pending
                            attend_finish(ph, pqb, pav, psacc)
                            if ph == NH - 1:
                                for j in range(SBK // P):
                                    oproj(pqb * (SBK // P) + j)
                        pending = (qb, h, st_[0], st_[1])
                    pqb, ph, pav, psacc = pending
                    attend_finish(ph, pqb, pav, psacc)
                    for j in range(SBK // P):
                        oproj(pqb * (SBK // P) + j)

            def body():
                resqk_cm = tc.tile_pool(name="resqk", bufs=1)
                resqk = resqk_cm.__enter__()
                qt = [resqk.tile([P, S], F32R, name=f"qt{h}") for h in range(NH)]
                kt_ = [resqk.tile([P, S], F32R, name=f"kt{h}") for h in range(NH)]
                phase_a(qt, kt_)
                resv_cm = tc.tile_pool(name="resv", bufs=1)
                resv = resv_cm.__enter__()
                vt = [resv.tile([P, DKL], F32R, name=f"vt{st}") for st in range(NST)]
                phase_b(vt)
                oto_cm = tc.tile_pool(name="oto", bufs=1)
                oto = oto_cm.__enter__()
                ot = [oto.tile([P, S], F32R, name=f"ot{h}") for h in range(NH)]
                phase_cd(qt, kt_, vt, ot)
                oto_cm.__exit__(None, None, None)
                resv_cm.__exit__(None, None, None)
                resqk_cm.__exit__(None, None, None)

            if repeat == 1:
                body()
            else:
                with tc.For_i(0, repeat, 1) as _i:
                    body()

    nc.compile()
    return nc


def _host_prep(x, W_Q, W_K, W_V, W_O, token_positions):
    x = np.asarray(x, dtype=np.float32)
    W_Q = np.asarray(W_Q, dtype=np.float32)
    W_K = np.asarray(W_K, dtype=np.float32)
    W_V = np.asarray(W_V, dtype=np.float32)
    W_O = np.asarray(W_O, dtype=np.float32)
    pos = np.asarray(token_positions).astype(np.float64)

    half = DK // 2
    inv_freq = THETA ** (-(np.arange(half, dtype=np.float64) / half))
    ang = pos[:, None] * inv_freq[None, :]          # [S, half]
    cosT = np.cos(ang).T.astype(np.float32)          # [64, S]
    sinT = np.sin(ang).T.astype(np.float32)
    cs1 = np.ascontiguousarray(np.concatenate([cosT, cosT], axis=0))   # [128, S]
    cs2 = np.ascontiguousarray(np.concatenate([-sinT, sinT], axis=0))

    # causal masks for the 4 diagonal-straddling offsets r: [128, 4*512]
    kk = np.arange(P)[:, None]
    qq = np.arange(SBK)[None, :]
    masks = np.concatenate(
        [(qq >= kk + P * r).astype(np.float32) for r in range(NH)], axis=1)
    masks = np.ascontiguousarray(masks)

    # deinterleave permutation within each head's 128 output dims
    perm = np.concatenate([np.arange(0, DK, 2), np.arange(1, DK, 2)])

    in_maps = []
    for c in range(N_CORES):
        b, hg = c // NH, c % NH
        hsl = slice(hg * DKL, (hg + 1) * DKL)
        wq = W_Q[hsl, :].reshape(NH, DK, D)[:, perm, :].reshape(DKL, D)
        wk = W_K[hsl, :].reshape(NH, DK, D)[:, perm, :].reshape(DKL, D)
        in_maps.append({
            "xT": np.ascontiguousarray(x[b].T),
            "wqT": np.ascontiguousarray(wq.T),
            "wkT": np.ascontiguousarray(wk.T),
            "wvT": np.ascontiguousarray(W_V[hsl, :].T),
            "woT": np.ascontiguousarray(W_O[:, hsl].T),
            "cs1": cs1,
            "cs2": cs2,
            "masks": masks,
        })
    return in_maps


def kernel(x, W_Q, W_K, W_V, W_O, token_positions):
    if "nc" not in _CACHE:
        _CACHE["nc"] = build_program()
    nc = _CACHE["nc"]
    in_maps = _host_prep(x, W_Q, W_K, W_V, W_O, token_positions)
    res = run_bass_kernel_spmd(nc, in_maps, list(range(N_CORES)))
    out = np.zeros((B, S, D), dtype=np.float32)
    for c in range(N_CORES):
        out[c // NH] += res.results[c]["out"]
    return out



# revision 2
# speedup vs baseline: 35.5452x; 35.5452x over previous
"""Causal multi-head self-attention (B=2, S=2048, D=2048, H=16) on 8 TRN2
NeuronCores.

Sharding: core c -> (batch b = c // 4, head-group hg = c % 4). Each core
computes 4 heads of one batch: QKV projections (tensor-parallel column
slices), RoPE, causal attention, and a partial W_O row-slice projection.
The host sums the 4 partial outputs per batch (replaces the all-reduce).

Layouts (per core):
  xT   [D, S]    x[b] transposed; matmul moving operand / stationary for V
  wqT  [D, 512]  W_Q[hslice].T with per-head deinterleave column permutation
  wkT  [D, 512]  same for W_K
  wvT  [D, 512]  W_V[hslice].T (natural order)
  woT  [512, D]  W_O[:, hslice].T (natural order)
  QT/KT per head [128, S] (transposed, deinterleaved dk order, RoPE applied)
  V per s-tile   [128, 512] (natural [s, dk] order)
  scores computed transposed [k, q] so exp tiles feed the AV matmul as the
  moving operand with V tiles stationary; softmax denominators via DVE
  accumulation + a ones-matmul that also broadcasts across partitions.

All matmuls use float32r (full-rate fp32 streaming) with N=512.
Projection phases run D-tile-outer: one xT tile streams through 8 (Q+K)
PSUM accumulation chains so only a few xT tiles are SBUF-live at a time.
"""
import sys

if "/opt/trn_rl_repo" not in sys.path:
    sys.path.insert(0, "/opt/trn_rl_repo")

import math
import numpy as np

import concourse.bass as bass
import concourse.mybir as mybir
import concourse.tile as tile
from concourse import bacc
from concourse.bass_utils import run_bass_kernel_spmd

B, S, D, H = 2, 2048, 2048, 16
DK = D // H            # 128
THETA = 10000.0
N_CORES = 8
NH = 4                 # heads per core
DKL = NH * DK          # 512 local head dims
P = 128
SBK = 512              # s-block (matmul N)
NDT = D // P           # 16 D-tiles
NST = S // P           # 16 s-tiles
NSB = S // SBK         # 4 s-blocks
NQB = S // SBK         # 4 q-blocks per head
NKT = S // P           # 16 k-tiles

F32 = mybir.dt.float32
F32R = mybir.dt.float32r

_CACHE = {}


def build_program(repeat=1):
    """Build the single-core SPMD program. repeat>1 wraps the body in a
    hardware loop (timing only)."""
    nc = bacc.Bacc("TRN2", target_bir_lowering=False, debug=False)

    xT = nc.dram_tensor("xT", [D, S], F32R, kind="ExternalInput").ap()
    wqT = nc.dram_tensor("wqT", [D, DKL], F32R, kind="ExternalInput").ap()
    wkT = nc.dram_tensor("wkT", [D, DKL], F32R, kind="ExternalInput").ap()
    wvT = nc.dram_tensor("wvT", [D, DKL], F32R, kind="ExternalInput").ap()
    woT = nc.dram_tensor("woT", [DKL, D], F32R, kind="ExternalInput").ap()
    cs1 = nc.dram_tensor("cs1", [P, S], F32, kind="ExternalInput").ap()
    cs2 = nc.dram_tensor("cs2", [P, S], F32, kind="ExternalInput").ap()
    masks = nc.dram_tensor("masks", [P, NH * SBK], F32, kind="ExternalInput").ap()
    out = nc.dram_tensor("out", [S, D], F32, kind="ExternalOutput").ap()

    inv_sqrt_dk = 1.0 / math.sqrt(DK)

    with tile.TileContext(nc) as tc:
        if True:

            def phase_a(qt, kt_):
                xTr = xT.rearrange("(dt p) s -> p dt s", p=P)
                with tc.tile_pool(name="wqk", bufs=1) as wqk, \
                     tc.tile_pool(name="csp", bufs=2) as csp, \
                     tc.tile_pool(name="xta", bufs=2) as xtp, \
                     tc.tile_pool(name="ropet", bufs=2) as ropep, \
                     tc.tile_pool(name="psa", bufs=8, space="PSUM") as psa:
                    wq_t = wqk.tile([P, NDT, DKL], F32R, name="wqs")
                    wk_t = wqk.tile([P, NDT, DKL], F32R, name="wks")
                    wqTr = wqT.rearrange("(dt p) n -> p dt n", p=P)
                    wkTr = wkT.rearrange("(dt p) n -> p dt n", p=P)
                    for sb in range(NSB):
                        cs1_t = csp.tile([P, SBK], F32, tag="cs1", name=f"cs1_{sb}")
                        cs2_t = csp.tile([P, SBK], F32, tag="cs2", name=f"cs2_{sb}")
                        pps = [psa.tile([P, SBK], F32, tag="pa", name=f"pa{sb}_{i}")
                               for i in range(2 * NH)]
                        for half in range(2):
                            hsl = slice(half * (NDT // 2), (half + 1) * (NDT // 2))
                            xs = xtp.tile([P, NDT // 2, SBK], F32R, tag="xt",
                                          name=f"xa{sb}_{half}")
                            nc.sync.dma_start(
                                out=xs,
                                in_=xTr[:, hsl, sb * SBK:(sb + 1) * SBK])
                            if sb == 0:
                                nc.sync.dma_start(out=wq_t[:, hsl, :], in_=wqTr[:, hsl, :])
                                nc.sync.dma_start(out=wk_t[:, hsl, :], in_=wkTr[:, hsl, :])
                            if half == 1:
                                nc.sync.dma_start(out=cs1_t, in_=cs1[:, sb * SBK:(sb + 1) * SBK])
                                nc.sync.dma_start(out=cs2_t, in_=cs2[:, sb * SBK:(sb + 1) * SBK])
                            for dd in range(NDT // 2):
                                d = half * (NDT // 2) + dd
                                for w_i, wt in enumerate((wq_t, wk_t)):
                                    for dkb in range(NH):
                                        nc.tensor.matmul(
                                            pps[w_i * NH + dkb][:],
                                            wt[:, d, dkb * P:(dkb + 1) * P], xs[:, dd, :],
                                            start=(d == 0), stop=(d == NDT - 1))
                        for w_i, dst in enumerate((qt, kt_)):
                            for dkb in range(NH):
                                pp = pps[w_i * NH + dkb]
                                # one ACT copy frees the PSUM bank; RoPE then
                                # reads SBUF only. rows 0:64 are x1, 64:128 x2.
                                qf = ropep.tile([P, SBK], F32, tag="qf")
                                nc.scalar.copy(qf[:], pp[:])
                                swp = ropep.tile([P, SBK], F32, tag="swp")
                                nc.scalar.copy(swp[0:P // 2], qf[P // 2:P])
                                nc.scalar.copy(swp[P // 2:P], qf[0:P // 2])
                                t1 = ropep.tile([P, SBK], F32, tag="t1")
                                nc.vector.tensor_mul(t1[:], qf[:], cs1_t[:])
                                t2 = ropep.tile([P, SBK], F32, tag="t2")
                                nc.vector.tensor_mul(t2[:], swp[:], cs2_t[:])
                                nc.vector.tensor_add(
                                    dst[dkb][:, sb * SBK:(sb + 1) * SBK], t1[:], t2[:])

            def phase_b(vt):
                xTr = xT.rearrange("(dt p) s -> p dt s", p=P)
                with tc.tile_pool(name="wvp", bufs=1) as wvp, \
                     tc.tile_pool(name="xtb", bufs=2) as xtpb, \
                     tc.tile_pool(name="psb", bufs=4, space="PSUM") as psb:
                    wv_t = wvp.tile([P, NDT, DKL], F32R, name="wvs")
                    wvTr = wvT.rearrange("(dt p) n -> p dt n", p=P)
                    for sb in range(NSB):
                        pvs = [psb.tile([P, DKL], F32, tag="pb", name=f"pb{sb}_{j}")
                               for j in range(SBK // P)]
                        for half in range(2):
                            hsl = slice(half * (NDT // 2), (half + 1) * (NDT // 2))
                            xs = xtpb.tile([P, NDT // 2, SBK], F32R, tag="xtb",
                                           name=f"xb{sb}_{half}")
                            nc.sync.dma_start(
                                out=xs,
                                in_=xTr[:, hsl, sb * SBK:(sb + 1) * SBK])
                            if sb == 0:
                                nc.sync.dma_start(out=wv_t[:, hsl, :], in_=wvTr[:, hsl, :])
                            for dd in range(NDT // 2):
                                d = half * (NDT // 2) + dd
                                for j in range(SBK // P):
                                    nc.tensor.matmul(
                                        pvs[j][:], xs[:, dd, j * P:(j + 1) * P], wv_t[:, d, :],
                                        start=(d == 0), stop=(d == NDT - 1))
                        for j in range(SBK // P):
                            st = sb * (SBK // P) + j
                            nc.scalar.copy(vt[st][:], pvs[j][:])

            def phase_cd(qt, kt_, vt, ot):
                # Attention (qb-outer, heads inner) interleaved with the
                # output projection for the finished q-block, so PE-dense
                # O-proj matmuls fill attention's ACT/DVE-bound stretches.
                with tc.tile_pool(name="maskp", bufs=1) as maskp, \
                     tc.tile_pool(name="wop", bufs=1) as wop, \
                     tc.tile_pool(name="expp", bufs=4) as expp, \
                     tc.tile_pool(name="saccp", bufs=2) as saccp, \
                     tc.tile_pool(name="recipp", bufs=2) as recipp, \
                     tc.tile_pool(name="stage", bufs=2) as stage, \
                     tc.tile_pool(name="pssc", bufs=2, space="PSUM") as pssc, \
                     tc.tile_pool(name="psav", bufs=2, space="PSUM") as psav, \
                     tc.tile_pool(name="psden", bufs=2, space="PSUM") as psden, \
                     tc.tile_pool(name="pso", bufs=2, space="PSUM") as pso:
                    mask_t = maskp.tile([P, NH * SBK], F32)
                    nc.sync.dma_start(out=mask_t, in_=masks)
                    ones_f = maskp.tile([P, P], F32)
                    nc.vector.memset(ones_f, 1.0)
                    ones_r = maskp.tile([P, P], F32R)
                    nc.vector.tensor_copy(ones_r, ones_f)
                    wo_t = wop.tile([P, NH, S], F32R, name="wos")
                    nc.sync.dma_start(out=wo_t, in_=woT.rearrange("(hh p) n -> p hh n", p=P))

                    def attend_main(h, qb):
                        nkt = 4 * qb + 4
                        av = psav.tile([P, SBK], F32, tag="av", name=f"av{h}_{qb}")
                        sacc = saccp.tile([P, SBK], F32R, tag="sacc", name=f"sacc{h}_{qb}")
                        prev = None
                        for kt in range(nkt):
                            sc = pssc.tile([P, SBK], F32, tag="sc", name=f"sc{h}_{qb}_{kt}")
                            nc.tensor.matmul(
                                sc[:], kt_[h][:, kt * P:(kt + 1) * P],
                                qt[h][:, qb * SBK:(qb + 1) * SBK],
                                start=True, stop=True)
                            e = expp.tile([P, SBK], F32R, tag="e", name=f"e{h}_{qb}_{kt}")
                            nc.scalar.activation(
                                e[:], sc[:], mybir.ActivationFunctionType.Exp,
                                scale=inv_sqrt_dk)
                            r = kt - 4 * qb
                            if r >= 0:  # diagonal-straddling tile: mask
                                em = expp.tile([P, SBK], F32R, tag="e", name=f"em{h}_{qb}_{kt}")
                                nc.vector.tensor_mul(
                                    em[:], e[:], mask_t[:, r * SBK:(r + 1) * SBK])
                                e = em
                            if kt == 0:
                                nc.vector.tensor_copy(sacc[:], e[:])
                            else:
                                nc.vector.tensor_add(sacc[:], sacc[:], e[:])
                            if prev is not None:
                                pkt, pe = prev
                                nc.tensor.matmul(
                                    av[:], vt[pkt][:, h * P:(h + 1) * P], pe[:],
                                    start=(pkt == 0), stop=False)
                            prev = (kt, e)
                        pkt, pe = prev
                        nc.tensor.matmul(
                            av[:], vt[pkt][:, h * P:(h + 1) * P], pe[:],
                            start=(pkt == 0), stop=True)
                        return av, sacc

                    def attend_finish(h, qb, av, sacc):
                        den = psden.tile([P, SBK], F32, tag="den", name=f"den{h}_{qb}")
                        nc.tensor.matmul(den[:], ones_r[:], sacc[:],
                                         start=True, stop=True)
                        recip = recipp.tile([P, SBK], F32, tag="recip", name=f"rc{h}_{qb}")
                        nc.vector.reciprocal_approx_fast(out=recip[:], in_=den[:])
                        nc.vector.tensor_mul(
                            ot[h][:, qb * SBK:(qb + 1) * SBK], av[:], recip[:])

                    def oproj(st):
                        for db in range(NSB):
                            po = pso.tile([P, SBK], F32, tag="po", name=f"po{st}_{db}")
                            for hh in range(NH):
                                nc.tensor.matmul(
                                    po[:], ot[hh][:, st * P:(st + 1) * P],
                                    wo_t[:, hh, db * SBK:(db + 1) * SBK],
                                    start=(hh == 0), stop=(hh == NH - 1))
                            og = stage.tile([P, SBK], F32, tag="og", name=f"og{st}_{db}")
                            nc.scalar.copy(og[:], po[:])
                            nc.sync.dma_start(
                                out=out[st * P:(st + 1) * P, db * SBK:(db + 1) * SBK],
                                in_=og[:])

                    # one-unit software pipeline: finish(u-1) emitted after
                    # main(u) so PE never stalls on the sacc chain, and the
                    # q-block's O-projection fills attention's ACT/DVE stretches.
                    units = [(qb, h) for qb in range(NQB) for h in range(NH)]
                    pending = None
                    for qb, h in units:
                        st_ = attend_main(h, qb)
                        if pending is not None:
                            pqb, ph, pav, psacc = pending
                            attend_finish(ph, pqb, pav, psacc)
                            if ph == NH - 1:
                                for j in range(SBK // P):
                                    oproj(pqb * (SBK // P) + j)
                        pending = (qb, h, st_[0], st_[1])
                    pqb, ph, pav, psacc = pending
                    attend_finish(ph, pqb, pav, psacc)
                    for j in range(SBK // P):
                        oproj(pqb * (SBK // P) + j)

            def body():
                resqk_cm = tc.tile_pool(name="resqk", bufs=1)
                resqk = resqk_cm.__enter__()
                qt = [resqk.tile([P, S], F32R, name=f"qt{h}") for h in range(NH)]
                kt_ = [resqk.tile([P, S], F32R, name=f"kt{h}") for h in range(NH)]
                with tc.spectator_scope("phaseA"):
                    phase_a(qt, kt_)
                resv_cm = tc.tile_pool(name="resv", bufs=1)
                resv = resv_cm.__enter__()
                vt = [resv.tile([P, DKL], F32R, name=f"vt{st}") for st in range(NST)]
                with tc.spectator_scope("phaseB"):
                    phase_b(vt)
                oto_cm = tc.tile_pool(name="oto", bufs=1)
                oto = oto_cm.__enter__()
                ot = [oto.tile([P, S], F32R, name=f"ot{h}") for h in range(NH)]
                with tc.spectator_scope("phaseCD"):
                    phase_cd(qt, kt_, vt, ot)
                oto_cm.__exit__(None, None, None)
                resv_cm.__exit__(None, None, None)
                resqk_cm.__exit__(None, None, None)

            if repeat == 1:
                body()
            else:
                with tc.For_i(0, repeat, 1) as _i:
                    body()

    nc.compile()
    return nc


def _host_prep(x, W_Q, W_K, W_V, W_O, token_positions):
    x = np.asarray(x, dtype=np.float32)
    W_Q = np.asarray(W_Q, dtype=np.float32)
    W_K = np.asarray(W_K, dtype=np.float32)
    W_V = np.asarray(W_V, dtype=np.float32)
    W_O = np.asarray(W_O, dtype=np.float32)
    pos = np.asarray(token_positions).astype(np.float64)

    half = DK // 2
    inv_freq = THETA ** (-(np.arange(half, dtype=np.float64) / half))
    ang = pos[:, None] * inv_freq[None, :]          # [S, half]
    cosT = np.cos(ang).T.astype(np.float32)          # [64, S]
    sinT = np.sin(ang).T.astype(np.float32)
    cs1 = np.ascontiguousarray(np.concatenate([cosT, cosT], axis=0))   # [128, S]
    cs2 = np.ascontiguousarray(np.concatenate([-sinT, sinT], axis=0))

    # causal masks for the 4 diagonal-straddling offsets r: [128, 4*512]
    kk = np.arange(P)[:, None]
    qq = np.arange(SBK)[None, :]
    masks = np.concatenate(
        [(qq >= kk + P * r).astype(np.float32) for r in range(NH)], axis=1)
    masks = np.ascontiguousarray(masks)

    # deinterleave permutation within each head's 128 output dims
    perm = np.concatenate([np.arange(0, DK, 2), np.arange(1, DK, 2)])

    in_maps = []
    for c in range(N_CORES):
        b, hg = c // NH, c % NH
        hsl = slice(hg * DKL, (hg + 1) * DKL)
        wq = W_Q[hsl, :].reshape(NH, DK, D)[:, perm, :].reshape(DKL, D)
        wk = W_K[hsl, :].reshape(NH, DK, D)[:, perm, :].reshape(DKL, D)
        in_maps.append({
            "xT": np.ascontiguousarray(x[b].T),
            "wqT": np.ascontiguousarray(wq.T),
            "wkT": np.ascontiguousarray(wk.T),
            "wvT": np.ascontiguousarray(W_V[hsl, :].T),
            "woT": np.ascontiguousarray(W_O[:, hsl].T),
            "cs1": cs1,
            "cs2": cs2,
            "masks": masks,
        })
    return in_maps


def kernel(x, W_Q, W_K, W_V, W_O, token_positions):
    if "nc" not in _CACHE:
        _CACHE["nc"] = build_program()
    nc = _CACHE["nc"]
    in_maps = _host_prep(x, W_Q, W_K, W_V, W_O, token_positions)
    res = run_bass_kernel_spmd(nc, in_maps, list(range(N_CORES)))
    out = np.zeros((B, S, D), dtype=np.float32)
    for c in range(N_CORES):
        out[c // NH] += res.results[c]["out"]
    return out

